# revision 1
# baseline (speedup 1.0000x reference)
"""Trainium2 Bass kernel: 16-head self-attention block (B=8, N=1024, C=1024).

Data-parallel over batch: each of the 8 NeuronCores processes one batch
element end-to-end (QKV proj -> attention -> softmax -> out proj). No
collectives needed. Compute in bf16 (fp32 PSUM accumulation).

Layout strategy per core:
  - x is only ever needed transposed (xT [c, n]); produced by chunked DMA +
    PE identity-matmul transposes, cast to bf16 on the PSUM->SBUF copy.
  - qT/kT computed per head-pair [128 = 2*Dh partitions, n] via
    w-stationary matmuls (rhs = xT).  Softmax scale folded into qT.
  - scores^T [m, n] via row-packed matmuls (two K=64 heads occupy array
    rows 0:63 / 64:127 concurrently); exp on ACT (no max subtraction --
    scores are O(1) by construction).
  - v stored natural [m, j] padded to 65 cols per head with a ones column:
    A.V matmul (lhsT = v|ones, M=65) accumulates out^T rows 0:63 and the
    softmax denominator in row 64 of the same PSUM tile.
  - normalization: 1/s = exp(-ln(s)) on ACT (shares one table set with the
    softmax exp), broadcast across partitions with a full-K selector
    matmul, DVE multiply into outT.
  - proj: outT-stationary matmuls, bias add, DMA out.
  - engines run their instruction streams in program order, so next-pair
    qkT groups and v/x chunks are explicitly interleaved into the
    ACT-paced attention loop to keep the PE and ACT both busy.

Measured on trn2 (8 cores): ~385 us NEFF exec, rel err ~4.6e-3 vs the
fp32 reference (bf16 compute, fp32 PSUM accumulation).
"""

import sys

sys.path.insert(0, "/opt/trn_rl_repo")

import numpy as np

P = 128
N = 1024  # tokens
C = 1024  # channels
H = 16  # heads
DH = 64  # head dim
NPAIR = 8  # head pairs
CO = C // P  # 8 outer chunks of contraction dim
NO = N // P  # 8 outer chunks of token dim
SCALE = DH ** -0.5
KERNEL_VERSION = 17  # bump on every semantic change (busts stale NEFF caches)

_CACHE = {}


def build_nc(dbg=False):
    import concourse.bass as bass
    import concourse.tile as tile
    from concourse import bacc, masks, mybir

    # Route Exp to natural_log_exp_and_others (which also holds Ln) so the
    # exp(-ln(s)) reciprocal shares one ACT table set with the softmax exp.
    if not getattr(bacc, "_exp_ln_patch", False):
        _orig_tables = bacc.get_activation_tables

        def _patched_tables(arch):
            t = _orig_tables(arch)
            for name, fns in t.items():
                if name != "natural_log_exp_and_others":
                    fns.discard(mybir.ActivationFunctionType.Exp)
            return t

        bacc.get_activation_tables = _patched_tables
        bacc._exp_ln_patch = True

    f32 = mybir.dt.float32
    bf16 = mybir.dt.bfloat16
    EXP = mybir.ActivationFunctionType.Exp
    LN = mybir.ActivationFunctionType.Ln

    nc = bacc.Bacc(None, target_bir_lowering=False)

    x_ext = nc.declare_dram_parameter("x", [N, C], f32, isOutput=False)
    wqkv_ext = nc.declare_dram_parameter("qkv_w", [C, 3 * C], f32, isOutput=False)
    wproj_ext = nc.declare_dram_parameter("proj_w", [C, C], f32, isOutput=False)
    pb_ext = nc.declare_dram_parameter("proj_b", [C], f32, isOutput=False)
    out_ext = nc.declare_dram_parameter("out", [N, C], f32, isOutput=True)
    # tiny version-stamped output: busts any executable cache keyed on the
    # HLO signature, and lets the harness confirm which kernel build ran
    ver_ext = nc.declare_dram_parameter(
        "kver", [1, KERNEL_VERSION], f32, isOutput=True
    )
    if dbg:
        dq_ext = nc.declare_dram_parameter("dq", [P, NPAIR, N], bf16, isOutput=True)
        dk_ext = nc.declare_dram_parameter("dk", [P, NPAIR, N], bf16, isOutput=True)
        dv_ext = nc.declare_dram_parameter(
            "dv", [P, NO, H, DH + 1], bf16, isOutput=True
        )
        dpt_ext = nc.declare_dram_parameter("dpt", [P, NO, N], bf16, isOutput=True)
        dot_ext = nc.declare_dram_parameter("dot", [P, NPAIR, N], bf16, isOutput=True)

    with tile.TileContext(nc) as tc:
        with (
            tc.tile_pool(name="big", bufs=1) as big,
            tc.tile_pool(name="work", bufs=3) as work,
            tc.tile_pool(name="ptp", bufs=4) as ptp,
            tc.tile_pool(name="mmp", bufs=2, space="PSUM") as mmp,
            tc.tile_pool(name="spool", bufs=2, space="PSUM") as spool,
            tc.tile_pool(name="avp", bufs=1, space="PSUM") as avp,
        ):
            # ---------------- constants / big buffers ----------------
            wq = big.tile([P, CO, C], bf16, tag="wq")
            wk = big.tile([P, CO, C], bf16, tag="wk")
            wv = big.tile([P, CO, C], bf16, tag="wv")
            wproj = big.tile([P, CO, C], bf16, tag="wproj")
            pb = big.tile([P, C], f32, tag="pb")
            xTs = [
                big.tile([P, N], bf16, tag=f"xT{co}", name=f"xT{co}")
                for co in range(CO)
            ]
            v_all = big.tile([P, NO, H, DH + 1], bf16, tag="v_all")
            qT = big.tile([P, NPAIR, N], bf16, tag="qT")
            kT = big.tile([P, NPAIR, N], bf16, tag="kT")
            outT = big.tile([P, NPAIR, N], bf16, tag="outT")
            stage_odd = big.tile([DH, NPAIR, N], bf16, tag="stage_odd")
            ident = big.tile([P, P], f32, tag="ident")
            # selector for the partition-broadcast matmul: row 64 ones
            sel_t = big.tile([P, DH], bf16, tag="sel_t")
            # persistent reciprocal row staging (rows 65:127 stay at 1.0 so
            # the K=64 broadcast matmul never touches uninitialized data)
            rec_t = big.tile([P, 512], bf16, tag="rec_t")

            ver_sb = big.tile([1, KERNEL_VERSION], f32, tag="ver_sb")
            nc.vector.memset(ver_sb, float(KERNEL_VERSION))
            nc.sync.dma_start(out=ver_ext[:, :], in_=ver_sb)
            # ones column of v|ones
            nc.vector.memset(v_all[:, :, :, DH : DH + 1], 1.0)
            nc.vector.memset(sel_t, 0.0)
            nc.vector.memset(sel_t[DH : DH + 1, :], 1.0)
            nc.vector.memset(rec_t, 1.0)
            masks.make_identity(nc, ident)

            # ---------------- input DMAs ----------------
            # weights (cast f32 -> bf16 during DMA); q and k first so the
            # first qkT matmuls can start as early as possible
            wqkv_src = wqkv_ext[:, :].rearrange("(o p) j -> p o j", p=P)
            nc.gpsimd.dma_start(out=wq[:, :, 0:P], in_=wqkv_src[:, :, 0:P])
            nc.gpsimd.dma_start(
                out=wk[:, :, 0:P], in_=wqkv_src[:, :, C : C + P]
            )
            # v weights, first half needed from attention(0)'s first A.V
            nc.gpsimd.dma_start(
                out=wv[:, :, 0:512], in_=wqkv_src[:, :, 2 * C : 2 * C + 512]
            )

            # x: per token chunk, DMA f32 rows then transpose on the PE
            # (fp32 identity-matmul transpose), cast to bf16 on the copy out
            def x_chunk(no):
                xf = work.tile([P, C], f32, tag="xf", name="xf")
                nc.sync.dma_start(out=xf, in_=x_ext[no * P : (no + 1) * P, :])
                for co in range(CO):
                    pst = mmp.tile([P, P], f32, tag="mm", name="pst")
                    nc.tensor.transpose(pst, xf[:, co * P : (co + 1) * P], ident)
                    nc.vector.tensor_copy(
                        xTs[co][:, no * P : (no + 1) * P], pst
                    )

            for no in range(4):
                x_chunk(no)
            # bias broadcast across partitions
            pb_ap = pb_ext[:]
            pb_src = bass.AP(
                tensor=pb_ap.tensor,
                offset=pb_ap.offset,
                ap=[[0, P], pb_ap.ap[0]],
            )
            nc.gpsimd.dma_start(out=pb, in_=pb_src)

            # ---------------- helpers ----------------
            def qk_group(pair, which, nh):
                """One q^T/k^T half: 8 accumulating matmuls + copy-out."""
                w = wq if which == 0 else wk
                dst = qT if which == 0 else kT
                ps = mmp.tile([P, 512], f32, tag="mm", name="ps")
                for co in range(CO):
                    nc.tensor.matmul(
                        ps,
                        w[:, co, pair * P : (pair + 1) * P],
                        xTs[co][:, nh * 512 : (nh + 1) * 512],
                        start=(co == 0),
                        stop=(co == CO - 1),
                    )
                if which == 0:
                    # fold softmax scale into q
                    nc.vector.tensor_scalar_mul(
                        dst[:, pair, nh * 512 : (nh + 1) * 512], ps, SCALE
                    )
                else:
                    nc.vector.tensor_copy(
                        dst[:, pair, nh * 512 : (nh + 1) * 512], ps
                    )

            def v_half(no, jh):
                """v columns for heads jh*8..jh*8+8, token chunk no."""
                ps = mmp.tile([P, 512], f32, tag="mm", name="ps")
                for co in range(CO):
                    nc.tensor.matmul(
                        ps,
                        xTs[co][:, no * P : (no + 1) * P],
                        wv[:, co, jh * 512 : (jh + 1) * 512],
                        start=(co == 0),
                        stop=(co == CO - 1),
                    )
                nc.vector.tensor_copy(
                    v_all[:, no, jh * 8 : (jh + 1) * 8, 0:DH],
                    ps[:].rearrange("p (h d) -> p h d", h=8),
                )

            def attention(pair, slot_fills=None):
                hA, hB = 2 * pair, 2 * pair + 1
                sf = slot_fills or {}
                for nh in range(2):
                    nsl = slice(nh * 512, (nh + 1) * 512)
                    avA = avp.tile([P, 512], f32, tag="avA")
                    avB = avp.tile([P, 512], f32, tag="avB")
                    for km in range(NO):
                        for fn in sf.get((nh, km), ()):
                            fn()
                        # scores^T for both heads: row-packed matmuls
                        s = spool.tile([P, N], f32, tag="S")
                        nc.tensor.matmul(
                            s[:, 0:512],
                            kT[0:DH, pair, km * P : (km + 1) * P],
                            qT[0:DH, pair, nsl],
                        )
                        nc.tensor.matmul(
                            s[:, 512:1024],
                            kT[DH:P, pair, km * P : (km + 1) * P],
                            qT[DH:P, pair, nsl],
                            tile_position=(DH, 0),
                        )
                        # exp (scores are O(1): no max subtraction needed)
                        pt = ptp.tile([P, N], bf16, tag="pt")
                        nc.scalar.activation(pt, s, EXP)
                        if dbg and pair == 0 and nh == 0:
                            nc.sync.dma_start(out=dpt_ext[:, km, :], in_=pt)
                        # A.V accumulation (ones column -> row 64 = denom)
                        nc.tensor.matmul(
                            avA[0 : DH + 1, :],
                            v_all[:, km, hA, :],
                            pt[:, 0:512],
                            start=(km == 0),
                            stop=(km == NO - 1),
                        )
                        nc.tensor.matmul(
                            avB[0 : DH + 1, :],
                            v_all[:, km, hB, :],
                            pt[:, 512:1024],
                            start=(km == 0),
                            stop=(km == NO - 1),
                        )
                    # epilogue: normalize by the accumulated denominator.
                    # 1/s computed as exp(-ln(s)) on ACT: shares the
                    # natural_log_exp table set with the softmax exp, and
                    # avoids the custom-DVE reciprocal (broken on HW here).
                    for head, av in ((hA, avA), (hB, avB)):
                        ln_row = work.tile([P, 512], f32, tag="ln_row")
                        nc.scalar.activation(
                            ln_row[DH : DH + 1, :], av[DH : DH + 1, :], LN
                        )
                        nc.scalar.activation(
                            rec_t[DH : DH + 1, :],
                            ln_row[DH : DH + 1, :],
                            EXP,
                            scale=-1.0,
                        )
                        # broadcast across partitions: full-K selector matmul
                        # (selector row 64 = 1; rec_t rows != 64 stay at 1.0)
                        bc = mmp.tile([DH, 512], f32, tag="mm")
                        nc.tensor.matmul(bc, sel_t, rec_t)
                        # DVE can't read two PSUM operands; stage bc in SBUF
                        bc_sb = work.tile([DH, 512], bf16, tag="bc_sb")
                        nc.vector.tensor_copy(bc_sb, bc)
                        if head % 2 == 0:
                            dst = outT[0:DH, pair, nsl]
                        else:
                            dst = stage_odd[:, pair, nsl]
                        nc.vector.tensor_mul(dst, av[0:DH, :], bc_sb)

            # ---------------- schedule ----------------
            def qkt_fill(pair):
                return [
                    lambda w=w, n=n: qk_group(pair, w, n)
                    for w in range(2)
                    for n in range(2)
                ]

            # n-half 0 of qT/kT only needs x chunks 0:4 -> start matmuls
            # while the remaining x chunks are still streaming in
            qk_group(0, 0, 0)
            qk_group(0, 1, 0)
            for no in range(4, NO):
                x_chunk(no)
            qk_group(0, 0, 1)
            qk_group(0, 1, 1)
            nc.gpsimd.dma_start(out=wq[:, :, P:C], in_=wqkv_src[:, :, P:C])
            nc.gpsimd.dma_start(
                out=wk[:, :, P:C], in_=wqkv_src[:, :, C + P : 2 * C]
            )
            nc.gpsimd.dma_start(
                out=wv[:, :, 512:1024],
                in_=wqkv_src[:, :, 2 * C + 512 : 3 * C],
            )
            def make_fills(pair):
                sf = {}
                if pair == 0:
                    # heads 0-7 v columns, just in time for each km
                    for km in range(NO):
                        sf[(0, km)] = [lambda k=km: v_half(k, 0)]
                    q = qkt_fill(1)
                    for i, km in enumerate((1, 3, 5, 7)):
                        sf[(1, km)] = [q[i]]
                elif pair in (1, 2):
                    # heads 8-15 v columns (needed from pair 4) + next qkT
                    q = qkt_fill(pair + 1)
                    vs = [
                        lambda k=k: v_half(k, 1)
                        for k in range((pair - 1) * 4, pair * 4)
                    ]
                    sf[(0, 0)] = [vs[0]]
                    sf[(0, 2)] = [q[0]]
                    sf[(0, 4)] = [vs[1]]
                    sf[(0, 6)] = [q[1]]
                    sf[(1, 0)] = [vs[2]]
                    sf[(1, 2)] = [q[2]]
                    sf[(1, 4)] = [vs[3]]
                    sf[(1, 6)] = [q[3]]
                elif pair < NPAIR - 1:
                    q = qkt_fill(pair + 1)
                    for i, s in enumerate(((0, 2), (0, 5), (1, 2), (1, 5))):
                        sf[s] = [q[i]]
                return sf

            for pair in range(NPAIR):
                if pair == 3:
                    # proj weights only needed at the tail; load mid-flight
                    nc.gpsimd.dma_start(
                        out=wproj,
                        in_=wproj_ext[:, :].rearrange("(o p) j -> p o j", p=P),
                    )
                attention(pair, slot_fills=make_fills(pair))
                # move this pair's odd head to partitions 64:128 right away
                nc.sync.dma_start(
                    out=outT[DH:P, pair, :], in_=stage_odd[:, pair, :]
                )

            if dbg:
                nc.sync.dma_start(out=dq_ext[:, :, :], in_=qT)
                nc.sync.dma_start(out=dk_ext[:, :, :], in_=kT)
                nc.sync.dma_start(out=dv_ext[:, :, :, :], in_=v_all)
                nc.sync.dma_start(out=dot_ext[:, :, :], in_=outT)

            # ---------------- output projection ----------------
            for no in range(NO):
                ps0 = mmp.tile([P, 512], f32, tag="mm", name="ps0")
                ps1 = mmp.tile([P, 512], f32, tag="mm", name="ps1")
                for pair in range(NPAIR):
                    lhs = outT[:, pair, no * P : (no + 1) * P]
                    nc.tensor.matmul(
                        ps0, lhs, wproj[:, pair, 0:512],
                        start=(pair == 0), stop=(pair == NPAIR - 1),
                    )
                    nc.tensor.matmul(
                        ps1, lhs, wproj[:, pair, 512:1024],
                        start=(pair == 0), stop=(pair == NPAIR - 1),
                    )
                for jh, ps in ((0, ps0), (1, ps1)):
                    res = work.tile([P, 512], f32, tag="res")
                    nc.vector.tensor_add(res, ps, pb[:, jh * 512 : (jh + 1) * 512])
                    nc.sync.dma_start(
                        out=out_ext[no * P : (no + 1) * P, jh * 512 : (jh + 1) * 512],
                        in_=res,
                    )

    nc.compile()
    return nc


def _get_nc():
    if "nc" not in _CACHE:
        _CACHE["nc"] = build_nc()
    return _CACHE["nc"]


def kernel(**inputs) -> np.ndarray:
    """Full-input entry point: shards batch over 8 cores, returns [8,N,C]."""
    from concourse.bass_utils import run_bass_kernel_spmd

    x = np.asarray(inputs["x"], dtype=np.float32)
    qkv_w = np.asarray(inputs["qkv_w"], dtype=np.float32)
    proj_w = np.asarray(inputs["proj_w"], dtype=np.float32)
    proj_b = np.asarray(inputs["proj_b"], dtype=np.float32)
    B = x.shape[0]
    assert B == 8, f"kernel hardcoded for B=8, got {B}"

    nc = _get_nc()
    in_maps = [
        {"x": x[i], "qkv_w": qkv_w, "proj_w": proj_w, "proj_b": proj_b}
        for i in range(B)
    ]
    res = run_bass_kernel_spmd(nc, in_maps, core_ids=list(range(B)))
    out = np.stack([res.results[i]["out"] for i in range(B)], axis=0)
    return out.astype(np.float32)



# revision 16
# speedup vs baseline: 1.0224x; 1.0224x over previous
"""Trainium2 Bass kernel: 16-head self-attention block (B=8, N=1024, C=1024).

Data-parallel over batch: each of the 8 NeuronCores processes one batch
element end-to-end (QKV proj -> attention -> softmax -> out proj). No
collectives. Compute in bf16 (fp32 PSUM accumulation).

v18 redesign vs v17 (382us): elastic fill scheduling. The attention inner
loop is ACT-paced (exp of [128,1024] scores = ~1.08us per km step) while
the PE only has ~0.65us of scores+AV work per step; v17 filled the slack
with coarse hand-placed qkT/v groups and still left ~62us of PE idle in
gaps. v18 keeps virtual PE/ACT clocks during emission and drains a fill
queue (remaining x-transposes, next pairs' qkT groups, v quarter-groups,
proj chunks) one matmul at a time whenever the PE clock falls behind.
Other changes: x DMA'd with f32->bf16 cast and transposed in bf16 (4
transposes batched per PSUM tile + one DVE copy), v computed in N=256
quarter groups JIT, per-nh stage_odd DMAs so the output projection of
token chunks 0-3 overlaps the last pair's second attention half.

Layout (unchanged from v17): xT via PE identity-transpose; qT/kT per
head-pair [128, n] w-stationary; scores^T row-packed (two K=64 heads
concurrent via tile_position); exp on ACT (scores O(1), no max-sub);
A.V with v|ones (denominator lands in PSUM row 64); 1/s = exp(-ln s) on
ACT sharing one table set; partition-broadcast via selector matmul.
"""

import sys

sys.path.insert(0, "/opt/trn_rl_repo")

from collections import deque

import numpy as np

P = 128
N = 1024  # tokens
C = 1024  # channels
H = 16  # heads
DH = 64  # head dim
NPAIR = 8  # head pairs
CO = C // P  # 8 outer chunks of contraction dim
NO = N // P  # 8 outer chunks of token dim
SCALE = DH ** -0.5
KERNEL_VERSION = 18  # bump on every semantic change (busts stale NEFF caches)

# virtual-clock cost estimates (ns) for the elastic scheduler
COST_MM512 = 216  # K<=128, N=512 bf16 matmul, stream-limited
COST_MM256 = 120
COST_TR = 170  # 128x128 bf16 transpose
COST_EXP = 1080  # ACT exp on [128,1024] f32->bf16
COST_EPI_ACT = 2700  # ACT ln+exp for both heads

_CACHE = {}


def build_nc():
    import concourse.bass as bass
    import concourse.tile as tile
    from concourse import bacc, masks, mybir

    # Route Exp to natural_log_exp_and_others (which also holds Ln) so the
    # exp(-ln(s)) reciprocal shares one ACT table set with the softmax exp.
    if not getattr(bacc, "_exp_ln_patch", False):
        _orig_tables = bacc.get_activation_tables

        def _patched_tables(arch):
            t = _orig_tables(arch)
            for name, fns in t.items():
                if name != "natural_log_exp_and_others":
                    fns.discard(mybir.ActivationFunctionType.Exp)
            return t

        bacc.get_activation_tables = _patched_tables
        bacc._exp_ln_patch = True

    f32 = mybir.dt.float32
    bf16 = mybir.dt.bfloat16
    EXP = mybir.ActivationFunctionType.Exp
    LN = mybir.ActivationFunctionType.Ln

    nc = bacc.Bacc(None, target_bir_lowering=False)

    x_ext = nc.declare_dram_parameter("x", [N, C], f32, isOutput=False)
    wqkv_ext = nc.declare_dram_parameter("qkv_w", [C, 3 * C], f32, isOutput=False)
    wproj_ext = nc.declare_dram_parameter("proj_w", [C, C], f32, isOutput=False)
    pb_ext = nc.declare_dram_parameter("proj_b", [C], f32, isOutput=False)
    out_ext = nc.declare_dram_parameter("out", [N, C], f32, isOutput=True)
    # tiny version-stamped output: busts any executable cache keyed on the
    # HLO signature, and lets the harness confirm which kernel build ran
    ver_ext = nc.declare_dram_parameter(
        "kver", [1, KERNEL_VERSION], f32, isOutput=True
    )

    with tile.TileContext(nc) as tc:
        with (
            tc.tile_pool(name="big", bufs=1) as big,
            tc.tile_pool(name="work", bufs=3) as work,
            tc.tile_pool(name="xbp", bufs=4) as xbp,
            tc.tile_pool(name="ptp", bufs=4) as ptp,
            tc.tile_pool(name="mmp", bufs=2, space="PSUM") as mmp,
            tc.tile_pool(name="spool", bufs=2, space="PSUM") as spool,
            tc.tile_pool(name="avp", bufs=1, space="PSUM") as avp,
        ):
            # ---------------- constants / big buffers ----------------
            wq = big.tile([P, CO, C], bf16, tag="wq")
            wk = big.tile([P, CO, C], bf16, tag="wk")
            wv = big.tile([P, CO, C], bf16, tag="wv")
            wproj = big.tile([P, CO, C], bf16, tag="wproj")
            pb = big.tile([P, C], f32, tag="pb")
            xT = big.tile([P, CO, N], bf16, tag="xT")
            v_all = big.tile([P, NO, H, DH + 1], bf16, tag="v_all")
            qT = big.tile([P, NPAIR, N], bf16, tag="qT")
            kT = big.tile([P, NPAIR, N], bf16, tag="kT")
            outT = big.tile([P, NPAIR, N], bf16, tag="outT")
            # single plane, reused per pair (DMA'd to outT within the pair)
            stage_odd = big.tile([DH, N], bf16, tag="stage_odd")
            ident = big.tile([P, P], bf16, tag="ident")
            # selector for the partition-broadcast matmul: row 64 ones
            sel_t = big.tile([P, DH], bf16, tag="sel_t")
            # persistent reciprocal staging, 2 slots (head A / head B) so the
            # two bc matmuls never WAR-stall the ACT; rows != 64 stay at 1.0
            # so the full-K broadcast matmul never touches uninitialized data
            rec_t = big.tile([P, 2, 512], bf16, tag="rec_t")

            ver_sb = big.tile([1, KERNEL_VERSION], f32, tag="ver_sb")
            nc.vector.memset(ver_sb, float(KERNEL_VERSION))
            nc.sync.dma_start(out=ver_ext[:, :], in_=ver_sb)
            # ones column of v|ones
            nc.vector.memset(v_all[:, :, :, DH : DH + 1], 1.0)
            nc.vector.memset(sel_t, 0.0)
            nc.vector.memset(sel_t[DH : DH + 1, :], 1.0)
            nc.vector.memset(rec_t, 1.0)
            masks.make_identity(nc, ident)

            # ---------------- input DMAs ----------------
            # weights (cast f32 -> bf16 during DMA); q and k first so the
            # first qkT matmuls can start as early as possible
            wqkv_src = wqkv_ext[:, :].rearrange("(o p) j -> p o j", p=P)
            nc.gpsimd.dma_start(out=wq[:, :, 0:P], in_=wqkv_src[:, :, 0:P])
            nc.gpsimd.dma_start(
                out=wk[:, :, 0:P], in_=wqkv_src[:, :, C : C + P]
            )
            # v weights, first half needed from attention(0)'s first A.V
            nc.gpsimd.dma_start(
                out=wv[:, :, 0:512], in_=wqkv_src[:, :, 2 * C : 2 * C + 512]
            )

            # x chunks: DMA with f32->bf16 cast, transpose in bf16 on the PE
            xbs = {}

            def x_dma(no):
                # 4-slot ring: chunk no+4's DMA waits until chunk no's
                # transposes consumed its slot (ramp does 0-3 eagerly)
                xb = xbp.tile([P, C], bf16, tag="xb")
                # f32 -> bf16 cast during DMA: gpsimd-initiated only
                nc.gpsimd.dma_start(
                    out=xb, in_=x_ext[no * P : (no + 1) * P, :]
                )
                xbs[no] = xb

            # bias broadcast across partitions
            pb_ap = pb_ext[:]
            pb_src = bass.AP(
                tensor=pb_ap.tensor,
                offset=pb_ap.offset,
                ap=[[0, P], pb_ap.ap[0]],
            )
            nc.gpsimd.dma_start(out=pb, in_=pb_src)

            # ---------------- elastic fill scheduler ----------------
            clocks = {"pe": 0.0, "act": 0.0}
            done = set()
            started = set()
            fillq = deque()  # (key, generator) — head-only draining

            def pe(ns):
                clocks["pe"] += ns

            def _step():
                """Advance the head fill unit by one instruction."""
                key, g = fillq[0]
                started.add(key)
                c = next(g, None)
                if c is None:
                    done.add(key)
                    fillq.popleft()
                    return 0.0
                pe(c)
                return c

            def drain(budget):
                spent = 0.0
                while fillq and spent < budget:
                    spent += _step()
                return spent

            def force(key):
                while key not in done:
                    _step()

            def close_open():
                """Finish a half-emitted fill unit so its mmp ring slots
                free in emission order (deadlock safety before bc/proj)."""
                if fillq and fillq[0][0] in started:
                    k = fillq[0][0]
                    force(k)

            def elastic(cap=1400.0):
                gap = clocks["act"] - clocks["pe"]
                if gap > 0:
                    drain(min(gap, cap))

            # ---------------- fill unit generators ----------------
            def g_transpose(no):
                """Transpose x chunk no: 2 PSUM groups of 4, 1 copy each."""
                xb = xbs[no]
                for g4 in range(2):
                    # transpose-mode out dtype must match lhsT dtype (bf16)
                    ps = mmp.tile([P, 512], bf16, tag="mm", name="pst")
                    for i in range(4):
                        co = g4 * 4 + i
                        nc.tensor.transpose(
                            ps[:, i * P : (i + 1) * P],
                            xb[:, co * P : (co + 1) * P],
                            ident,
                        )
                        if not (i == 3):
                            yield COST_TR
                    nc.vector.tensor_copy(
                        xT[:, g4 * 4 : g4 * 4 + 4, no * P : (no + 1) * P],
                        ps[:].rearrange("p (c q) -> p c q", c=4),
                    )
                    yield COST_TR

            def g_qk(pair, which, nh):
                """One q^T/k^T half: 8 accumulating matmuls + copy-out."""
                w = wq if which == 0 else wk
                dst = qT if which == 0 else kT
                nsl = slice(nh * 512, (nh + 1) * 512)
                ps = mmp.tile([P, 512], f32, tag="mm", name="ps")
                for co in range(CO):
                    nc.tensor.matmul(
                        ps,
                        w[:, co, pair * P : (pair + 1) * P],
                        xT[:, co, nsl],
                        start=(co == 0),
                        stop=(co == CO - 1),
                    )
                    if co < CO - 1:
                        yield COST_MM512
                if which == 0:
                    # fold softmax scale into q
                    nc.vector.tensor_scalar_mul(dst[:, pair, nsl], ps, SCALE)
                else:
                    nc.vector.tensor_copy(dst[:, pair, nsl], ps)
                yield COST_MM512

            def g_v(km, qt):
                """v columns for heads 4qt..4qt+4, token chunk km."""
                ps = mmp.tile([P, 256], f32, tag="mm", name="psv")
                for co in range(CO):
                    nc.tensor.matmul(
                        ps,
                        xT[:, co, km * P : (km + 1) * P],
                        wv[:, co, qt * 256 : (qt + 1) * 256],
                        start=(co == 0),
                        stop=(co == CO - 1),
                    )
                    if co < CO - 1:
                        yield COST_MM256
                nc.vector.tensor_copy(
                    v_all[:, km, qt * 4 : (qt + 1) * 4, 0:DH],
                    ps[:].rearrange("p (h d) -> p h d", h=4),
                )
                yield COST_MM256

            def g_proj(no):
                """Output projection for token chunk no (both jh halves)."""
                ps0 = mmp.tile([P, 512], f32, tag="mm", name="ps0")
                ps1 = mmp.tile([P, 512], f32, tag="mm", name="ps1")
                for pair in range(NPAIR):
                    lhs = outT[:, pair, no * P : (no + 1) * P]
                    nc.tensor.matmul(
                        ps0, lhs, wproj[:, pair, 0:512],
                        start=(pair == 0), stop=(pair == NPAIR - 1),
                    )
                    yield COST_MM512
                    nc.tensor.matmul(
                        ps1, lhs, wproj[:, pair, 512:1024],
                        start=(pair == 0), stop=(pair == NPAIR - 1),
                    )
                    if pair < NPAIR - 1:
                        yield COST_MM512
                for jh, ps in ((0, ps0), (1, ps1)):
                    res = work.tile([P, 512], f32, tag="res")
                    nc.vector.tensor_add(
                        res, ps, pb[:, jh * 512 : (jh + 1) * 512]
                    )
                    nc.sync.dma_start(
                        out=out_ext[
                            no * P : (no + 1) * P, jh * 512 : (jh + 1) * 512
                        ],
                        in_=res,
                    )
                yield COST_MM512

            # ---------------- attention ----------------
            def emit_S(pair, nh, km):
                """scores^T for both heads of `pair`: row-packed matmuls,
                then the exp on ACT. Returns the pt tile."""
                nsl = slice(nh * 512, (nh + 1) * 512)
                s = spool.tile([P, N], f32, tag="S")
                nc.tensor.matmul(
                    s[:, 0:512],
                    kT[0:DH, pair, km * P : (km + 1) * P],
                    qT[0:DH, pair, nsl],
                )
                nc.tensor.matmul(
                    s[:, 512:1024],
                    kT[DH:P, pair, km * P : (km + 1) * P],
                    qT[DH:P, pair, nsl],
                    tile_position=(DH, 0),
                )
                pe(COST_MM512)  # concurrent pair
                pt = ptp.tile([P, N], bf16, tag="pt")
                nc.scalar.activation(pt, s, EXP)
                clocks["act"] = max(clocks["act"], clocks["pe"]) + COST_EXP
                return pt

            def attention(pair):
                qt = pair // 2
                hA, hB = 2 * pair, 2 * pair + 1
                for nh in range(2):
                    nsl = slice(nh * 512, (nh + 1) * 512)
                    avA = avp.tile([P, 512], f32, tag="avA")
                    avB = avp.tile([P, 512], f32, tag="avB")
                    force(("qk", pair, 1, 0))
                    force(("qk", pair, 0, nh))
                    pts = {0: emit_S(pair, nh, 0)}
                    for km in range(NO):
                        exp_done = clocks["act"]  # exp(km) finish frontier
                        if km + 1 < NO:
                            if km + 1 == 4:
                                force(("qk", pair, 1, 1))
                            elastic()
                            pts[km + 1] = emit_S(pair, nh, km + 1)
                        force(("v", qt, km))
                        pt = pts.pop(km)
                        # A.V of km stalls until exp(km) is done
                        clocks["pe"] = max(clocks["pe"], exp_done)
                        nc.tensor.matmul(
                            avA[0 : DH + 1, :],
                            v_all[:, km, hA, :],
                            pt[:, 0:512],
                            start=(km == 0),
                            stop=(km == NO - 1),
                        )
                        nc.tensor.matmul(
                            avB[0 : DH + 1, :],
                            v_all[:, km, hB, :],
                            pt[:, 512:1024],
                            start=(km == 0),
                            stop=(km == NO - 1),
                        )
                        pe(2 * COST_MM512)
                    # epilogue: 1/denominator via exp(-ln) on ACT, broadcast
                    # across partitions with a full-K selector matmul, DVE mul
                    for slot, av in ((0, avA), (1, avB)):
                        ln_row = work.tile([P, 512], f32, tag="ln_row")
                        nc.scalar.activation(
                            ln_row[DH : DH + 1, :], av[DH : DH + 1, :], LN
                        )
                        nc.scalar.activation(
                            rec_t[DH : DH + 1, slot, :],
                            ln_row[DH : DH + 1, :],
                            EXP,
                            scale=-1.0,
                        )
                    clocks["act"] += COST_EPI_ACT
                    # fill until the reciprocal is ready, then finish any
                    # half-open unit so bc's mmp slot frees in order
                    elastic(1e9)
                    close_open()
                    clocks["pe"] = max(clocks["pe"], clocks["act"])
                    for slot, head, av in ((0, hA, avA), (1, hB, avB)):
                        bc = mmp.tile([DH, 512], f32, tag="mm", name="bc")
                        nc.tensor.matmul(bc, sel_t, rec_t[:, slot, :])
                        pe(COST_MM512)
                        # DVE can't read two PSUM operands; stage bc in SBUF
                        bc_sb = work.tile([DH, 512], bf16, tag="bc_sb")
                        nc.vector.tensor_copy(bc_sb, bc)
                        if head % 2 == 0:
                            dst = outT[0:DH, pair, nsl]
                        else:
                            dst = stage_odd[:, nsl]
                        nc.vector.tensor_mul(dst, av[0:DH, :], bc_sb)
                    # move this half's odd head to partitions 64:128 now so
                    # proj of these token chunks can start (pair 7 overlap)
                    nc.sync.dma_start(
                        out=outT[DH:P, pair, nsl],
                        in_=stage_odd[:, nsl],
                    )
                    if pair == NPAIR - 1:
                        # outT complete for token chunks of this half: the
                        # output projection becomes legal fill work
                        for no in range(nh * 4, nh * 4 + 4):
                            fillq.append((("proj", no), g_proj(no)))

            # ---------------- schedule ----------------
            # ramp: x chunks 0-3 -> transposes -> pair-0 nh0 qT/kT.
            # chunk no+4's DMA is emitted after chunk no's transposes so the
            # 4-slot ring's write order matches read order.
            for no in range(4):
                x_dma(no)
            for no in range(4):
                for c in g_transpose(no):
                    pe(c)
                x_dma(no + 4)
            for c in g_qk(0, 1, 0):
                pe(c)
            for c in g_qk(0, 0, 0):
                pe(c)
            done.update({("qk", 0, 1, 0), ("qk", 0, 0, 0)})

            # fill queue in dependency order (head-only draining keeps at
            # most one group open in the mmp ring -> no deadlock)
            for km in range(4):
                fillq.append((("v", 0, km), g_v(km, 0)))
            for no in range(4, NO):
                fillq.append((("tr", no), g_transpose(no)))
            for km in range(4, NO):
                fillq.append((("v", 0, km), g_v(km, 0)))
            fillq.append((("qk", 0, 1, 1), g_qk(0, 1, 1)))
            fillq.append((("qk", 0, 0, 1), g_qk(0, 0, 1)))

            def queue_pair_qk(p):
                for which in (1, 0):
                    for nh in range(2):
                        fillq.append(
                            ((("qk", p, which, nh)), g_qk(p, which, nh))
                        )

            queue_pair_qk(1)
            for km in range(NO):
                fillq.append((("v", 1, km), g_v(km, 1)))
            queue_pair_qk(2)
            queue_pair_qk(3)
            for km in range(NO):
                fillq.append((("v", 2, km), g_v(km, 2)))
            queue_pair_qk(4)
            queue_pair_qk(5)
            for km in range(NO):
                fillq.append((("v", 3, km), g_v(km, 3)))
            queue_pair_qk(6)
            queue_pair_qk(7)

            # remaining weight DMAs once the early ones are in flight
            nc.gpsimd.dma_start(out=wq[:, :, P:C], in_=wqkv_src[:, :, P:C])
            nc.gpsimd.dma_start(
                out=wk[:, :, P:C], in_=wqkv_src[:, :, C + P : 2 * C]
            )
            nc.gpsimd.dma_start(
                out=wv[:, :, 512:1024],
                in_=wqkv_src[:, :, 2 * C + 512 : 3 * C],
            )

            for pair in range(NPAIR):
                if pair == 3:
                    # proj weights only needed at the tail; load mid-flight
                    nc.gpsimd.dma_start(
                        out=wproj,
                        in_=wproj_ext[:, :].rearrange("(o p) j -> p o j", p=P),
                    )
                attention(pair)

            # tail: whatever fills remain (proj chunks 4-7)
            while fillq:
                drain(1e9)

    nc.compile()
    return nc


def _get_nc():
    if "nc" not in _CACHE:
        _CACHE["nc"] = build_nc()
    return _CACHE["nc"]


def kernel(**inputs) -> np.ndarray:
    """Full-input entry point: shards batch over 8 cores, returns [8,N,C]."""
    from concourse.bass_utils import run_bass_kernel_spmd

    x = np.asarray(inputs["x"], dtype=np.float32)
    qkv_w = np.asarray(inputs["qkv_w"], dtype=np.float32)
    proj_w = np.asarray(inputs["proj_w"], dtype=np.float32)
    proj_b = np.asarray(inputs["proj_b"], dtype=np.float32)
    B = x.shape[0]
    assert B == 8, f"kernel hardcoded for B=8, got {B}"

    nc = _get_nc()
    in_maps = [
        {"x": x[i], "qkv_w": qkv_w, "proj_w": proj_w, "proj_b": proj_b}
        for i in range(B)
    ]
    res = run_bass_kernel_spmd(nc, in_maps, core_ids=list(range(B)))
    out = np.stack([res.results[i]["out"] for i in range(B)], axis=0)
    return out.astype(np.float32)


# revision 31
# speedup vs baseline: 1.1848x; 1.1588x over previous
"""Trainium2 Bass kernel: 16-head self-attention block (B=8, N=1024, C=1024).

Data-parallel over batch: each of the 8 NeuronCores processes one batch
element end-to-end (QKV proj -> attention -> softmax -> out proj). No
collectives. Compute in bf16 (fp32 PSUM accumulation).

v19 (from v17 382us -> v18 379us): elastic fill scheduling with virtual
PE/ACT clocks — drains a fill queue (x-transposes, next pairs' qkT
groups, v quarter-groups, projection partials) one matmul at a time
whenever the PE clock falls behind the ACT clock. v18 traces showed the
dense schedule runs the whole chip in the P0 power state (PE 2.0 GHz,
ACT 1.0 GHz; N=512 matmul = 259 ns not 216), the ramp was DMA-starved
(x chunks queued behind 2 MB of wv on the gpsimd queue), fills ran out
during pairs 5-7, and the tail held the full output projection. v19:
costs calibrated to P0 clocks; gpsimd DMA queue reordered x-first;
fill rationing (per-unit earliest-pair tags) so late pairs keep supply;
projection split into a pairs-0-6 partial (fill work for pairs 6-7,
staged +bias in SBUF) and a tiny pair-7 remainder at the tail.
Other v18 changes kept: x DMA'd with f32->bf16 cast and transposed in
bf16 (4 transposes per PSUM tile + one DVE copy), v in N=256 quarter
groups JIT, per-nh stage_odd DMAs.

Layout (unchanged from v17): xT via PE identity-transpose; qT/kT per
head-pair [128, n] w-stationary; scores^T row-packed (two K=64 heads
concurrent via tile_position); exp on ACT (scores O(1), no max-sub);
A.V with v|ones (denominator lands in PSUM row 64); 1/s = exp(-ln s) on
ACT sharing one table set; partition-broadcast via selector matmul.
"""

import sys

sys.path.insert(0, "/opt/trn_rl_repo")

from collections import deque

import numpy as np

P = 128
N = 1024  # tokens
C = 1024  # channels
H = 16  # heads
DH = 64  # head dim
NPAIR = 8  # head pairs
CO = C // P  # 8 outer chunks of contraction dim
NO = N // P  # 8 outer chunks of token dim
SCALE = DH ** -0.5
KERNEL_VERSION = 19  # bump on every semantic change (busts stale NEFF caches)

# virtual-clock cost estimates (ns) for the elastic scheduler, calibrated
# on HW traces at the P0 power-state clocks the dense schedule runs at
# (PE ~2.0 GHz, ACT ~1.0 GHz): N=512 matmul streams at ~259 ns, the
# score pair pays an extra unhidden kT LDWEIGHTS (~390 ns total), exp of
# [128,1024] is ~1340 ns.
COST_MM512 = 260
COST_MM256 = 136
COST_TR = 128  # 128x128 bf16 transpose
COST_SPAIR = 390  # row-packed score pair incl. kT weight load
COST_AV = 260
COST_EXP = 1340  # ACT exp on [128,1024] f32->bf16
COST_EPI_ACT = 3350  # ACT ln+exp for both heads

_CACHE = {}


def build_nc():
    import concourse.bass as bass
    import concourse.tile as tile
    from concourse import bacc, masks, mybir

    # Route Exp to natural_log_exp_and_others (which also holds Ln) so the
    # exp(-ln(s)) reciprocal shares one ACT table set with the softmax exp.
    if not getattr(bacc, "_exp_ln_patch", False):
        _orig_tables = bacc.get_activation_tables

        def _patched_tables(arch):
            t = _orig_tables(arch)
            for name, fns in t.items():
                if name != "natural_log_exp_and_others":
                    fns.discard(mybir.ActivationFunctionType.Exp)
            return t

        bacc.get_activation_tables = _patched_tables
        bacc._exp_ln_patch = True

    f32 = mybir.dt.float32
    bf16 = mybir.dt.bfloat16
    EXP = mybir.ActivationFunctionType.Exp
    LN = mybir.ActivationFunctionType.Ln

    nc = bacc.Bacc(None, target_bir_lowering=False)

    x_ext = nc.declare_dram_parameter("x", [N, C], f32, isOutput=False)
    wqkv_ext = nc.declare_dram_parameter("qkv_w", [C, 3 * C], f32, isOutput=False)
    wproj_ext = nc.declare_dram_parameter("proj_w", [C, C], f32, isOutput=False)
    pb_ext = nc.declare_dram_parameter("proj_b", [C], f32, isOutput=False)
    out_ext = nc.declare_dram_parameter("out", [N, C], f32, isOutput=True)
    # tiny version-stamped output: busts any executable cache keyed on the
    # HLO signature, and lets the harness confirm which kernel build ran
    ver_ext = nc.declare_dram_parameter(
        "kver", [1, KERNEL_VERSION], f32, isOutput=True
    )

    with tile.TileContext(nc) as tc:
        with (
            tc.tile_pool(name="big", bufs=1) as big,
            tc.tile_pool(name="work", bufs=3) as work,
            tc.tile_pool(name="xbp", bufs=4) as xbp,
            tc.tile_pool(name="ptp", bufs=4) as ptp,
            tc.tile_pool(name="mmp", bufs=2, space="PSUM") as mmp,
            tc.tile_pool(name="spool", bufs=2, space="PSUM") as spool,
            tc.tile_pool(name="avp", bufs=1, space="PSUM") as avp,
        ):
            # ---------------- constants / big buffers ----------------
            wq = big.tile([P, CO, C], bf16, tag="wq")
            wk = big.tile([P, CO, C], bf16, tag="wk")
            wv = big.tile([P, CO, C], bf16, tag="wv")
            wproj = big.tile([P, CO, C], bf16, tag="wproj")
            pb = big.tile([P, C], f32, tag="pb")
            xT = big.tile([P, CO, N], bf16, tag="xT")
            v_all = big.tile([P, NO, H, DH + 1], bf16, tag="v_all")
            qT = big.tile([P, NPAIR, N], bf16, tag="qT")
            kT = big.tile([P, NPAIR, N], bf16, tag="kT")
            outT = big.tile([P, NPAIR, N], bf16, tag="outT")
            # single plane, reused per pair (DMA'd to outT within the pair)
            stage_odd = big.tile([DH, N], bf16, tag="stage_odd")
            ident = big.tile([P, P], bf16, tag="ident")
            # selector for the partition-broadcast matmul: row 64 ones
            sel_t = big.tile([P, DH], bf16, tag="sel_t")
            # persistent reciprocal staging, 2 slots (head A / head B) so the
            # two bc matmuls never WAR-stall the ACT; rows != 64 stay at 1.0
            # so the full-K broadcast matmul never touches uninitialized data
            rec_t = big.tile([P, 2, 512], bf16, tag="rec_t")
            # pairs 0-6 projection partials + bias, staged for the tail
            partial_sb = big.tile([P, NO, 2, 512], bf16, tag="partial_sb")

            ver_sb = big.tile([1, KERNEL_VERSION], f32, tag="ver_sb")
            nc.vector.memset(ver_sb, float(KERNEL_VERSION))
            nc.sync.dma_start(out=ver_ext[:, :], in_=ver_sb)
            # ones column of v|ones
            nc.vector.memset(v_all[:, :, :, DH : DH + 1], 1.0)
            nc.vector.memset(sel_t, 0.0)
            nc.vector.memset(sel_t[DH : DH + 1, :], 1.0)
            nc.vector.memset(rec_t, 1.0)
            masks.make_identity(nc, ident)

            # ---------------- input DMAs ----------------
            # all casting DMAs go through the gpsimd queue; order matters:
            # x chunks 0-3 first (they gate the whole ramp), then just the
            # weight slices the first matmuls need
            wqkv_src = wqkv_ext[:, :].rearrange("(o p) j -> p o j", p=P)

            # x chunks: DMA with f32->bf16 cast, transpose in bf16 on the PE
            xbs = {}

            def x_dma(no):
                # 4-slot ring: chunk no+4's DMA waits until chunk no's
                # transposes consumed its slot (ramp does 0-3 eagerly)
                xb = xbp.tile([P, C], bf16, tag="xb")
                # f32 -> bf16 cast during DMA: gpsimd-initiated only
                nc.gpsimd.dma_start(
                    out=xb, in_=x_ext[no * P : (no + 1) * P, :]
                )
                xbs[no] = xb

            # ---------------- elastic fill scheduler ----------------
            clocks = {"pe": 0.0, "act": 0.0}
            cur = {"pair": -1}
            done = set()
            started = set()
            fillq = deque()  # (key, generator, min_pair) — head-only drain

            def pe(ns):
                clocks["pe"] += ns

            def _step():
                """Advance the head fill unit by one instruction."""
                key, g, _tag = fillq[0]
                started.add(key)
                c = next(g, None)
                if c is None:
                    done.add(key)
                    fillq.popleft()
                    return 0.0
                pe(c)
                return c

            def drain(budget, respect_tags=True):
                """Drain fills; rationing: don't start a unit tagged for a
                later pair (keeps fill supply for late pairs), but always
                finish a unit already started."""
                spent = 0.0
                while fillq and spent < budget:
                    key, _g, tag = fillq[0]
                    if (
                        respect_tags
                        and key not in started
                        and tag > cur["pair"] + 1
                    ):
                        break
                    spent += _step()
                return spent

            def force(key):
                while key not in done:
                    _step()

            def close_open():
                """Finish a half-emitted fill unit so its mmp ring slots
                free in emission order (deadlock safety before bc/proj)."""
                if fillq and fillq[0][0] in started:
                    k = fillq[0][0]
                    force(k)

            def elastic(cap=1400.0):
                gap = clocks["act"] - clocks["pe"]
                if gap > 0:
                    drain(min(gap, cap))

            # ---------------- fill unit generators ----------------
            def g_transpose(no):
                """Transpose x chunk no: 2 PSUM groups of 4, 1 copy each."""
                xb = xbs[no]
                for g4 in range(2):
                    # transpose-mode out dtype must match lhsT dtype (bf16)
                    ps = mmp.tile([P, 512], bf16, tag="mm", name="pst")
                    for i in range(4):
                        co = g4 * 4 + i
                        nc.tensor.transpose(
                            ps[:, i * P : (i + 1) * P],
                            xb[:, co * P : (co + 1) * P],
                            ident,
                        )
                        if not (i == 3):
                            yield COST_TR
                    nc.vector.tensor_copy(
                        xT[:, g4 * 4 : g4 * 4 + 4, no * P : (no + 1) * P],
                        ps[:].rearrange("p (c q) -> p c q", c=4),
                    )
                    yield COST_TR

            def g_qk(pair, which, nh):
                """One q^T/k^T half: 8 accumulating matmuls + copy-out."""
                w = wq if which == 0 else wk
                dst = qT if which == 0 else kT
                nsl = slice(nh * 512, (nh + 1) * 512)
                ps = mmp.tile([P, 512], f32, tag="mm", name="ps")
                for co in range(CO):
                    nc.tensor.matmul(
                        ps,
                        w[:, co, pair * P : (pair + 1) * P],
                        xT[:, co, nsl],
                        start=(co == 0),
                        stop=(co == CO - 1),
                    )
                    if co < CO - 1:
                        yield COST_MM512
                if which == 0:
                    # fold softmax scale into q
                    nc.vector.tensor_scalar_mul(dst[:, pair, nsl], ps, SCALE)
                else:
                    nc.vector.tensor_copy(dst[:, pair, nsl], ps)
                yield COST_MM512

            def g_v(km, qt):
                """v columns for heads 4qt..4qt+4, token chunk km."""
                ps = mmp.tile([P, 256], f32, tag="mm", name="psv")
                for co in range(CO):
                    nc.tensor.matmul(
                        ps,
                        xT[:, co, km * P : (km + 1) * P],
                        wv[:, co, qt * 256 : (qt + 1) * 256],
                        start=(co == 0),
                        stop=(co == CO - 1),
                    )
                    if co < CO - 1:
                        yield COST_MM256
                nc.vector.tensor_copy(
                    v_all[:, km, qt * 4 : (qt + 1) * 4, 0:DH],
                    ps[:].rearrange("p (h d) -> p h d", h=4),
                )
                yield COST_MM256

            def g_partial(no):
                """Output projection for token chunk no, pairs 0-6 only.
                Runs as fill work once pairs 0-6's outT covers this chunk
                (mid pair-6); result + bias staged in SBUF so only pair 7's
                single accumulation step remains at the tail."""
                ps0 = mmp.tile([P, 512], f32, tag="mm", name="ps0")
                ps1 = mmp.tile([P, 512], f32, tag="mm", name="ps1")
                for pair in range(NPAIR - 1):
                    lhs = outT[:, pair, no * P : (no + 1) * P]
                    nc.tensor.matmul(
                        ps0, lhs, wproj[:, pair, 0:512],
                        start=(pair == 0), stop=(pair == NPAIR - 2),
                    )
                    yield COST_MM512
                    nc.tensor.matmul(
                        ps1, lhs, wproj[:, pair, 512:1024],
                        start=(pair == 0), stop=(pair == NPAIR - 2),
                    )
                    if pair < NPAIR - 2:
                        yield COST_MM512
                for jh, ps in ((0, ps0), (1, ps1)):
                    nc.vector.tensor_add(
                        partial_sb[:, no, jh, :],
                        ps,
                        pb[:, jh * 512 : (jh + 1) * 512],
                    )
                yield COST_MM512

            def g_remainder(no):
                """Tail of the output projection for chunk no: pair 7's
                contribution + staged partial, then DMA out."""
                ps0 = mmp.tile([P, 512], f32, tag="mm", name="ps0")
                ps1 = mmp.tile([P, 512], f32, tag="mm", name="ps1")
                lhs = outT[:, NPAIR - 1, no * P : (no + 1) * P]
                nc.tensor.matmul(ps0, lhs, wproj[:, NPAIR - 1, 0:512])
                yield COST_MM512
                nc.tensor.matmul(ps1, lhs, wproj[:, NPAIR - 1, 512:1024])
                for jh, ps in ((0, ps0), (1, ps1)):
                    res = work.tile([P, 512], f32, tag="res")
                    nc.vector.tensor_add(res, ps, partial_sb[:, no, jh, :])
                    nc.sync.dma_start(
                        out=out_ext[
                            no * P : (no + 1) * P, jh * 512 : (jh + 1) * 512
                        ],
                        in_=res,
                    )
                yield COST_MM512

            # ---------------- attention ----------------
            def emit_S(pair, nh, km):
                """scores^T for both heads of `pair`: row-packed matmuls,
                then the exp on ACT. Returns the pt tile."""
                nsl = slice(nh * 512, (nh + 1) * 512)
                s = spool.tile([P, N], f32, tag="S")
                nc.tensor.matmul(
                    s[:, 0:512],
                    kT[0:DH, pair, km * P : (km + 1) * P],
                    qT[0:DH, pair, nsl],
                )
                nc.tensor.matmul(
                    s[:, 512:1024],
                    kT[DH:P, pair, km * P : (km + 1) * P],
                    qT[DH:P, pair, nsl],
                    tile_position=(DH, 0),
                )
                pe(COST_SPAIR)  # concurrent pair + kT weight load
                pt = ptp.tile([P, N], bf16, tag="pt")
                nc.scalar.activation(pt, s, EXP)
                clocks["act"] = max(clocks["act"], clocks["pe"]) + COST_EXP
                return pt

            def attention(pair):
                cur["pair"] = pair
                qt = pair // 2
                hA, hB = 2 * pair, 2 * pair + 1
                for nh in range(2):
                    nsl = slice(nh * 512, (nh + 1) * 512)
                    avA = avp.tile([P, 512], f32, tag="avA")
                    avB = avp.tile([P, 512], f32, tag="avB")
                    force(("qk", pair, 1, 0))
                    force(("qk", pair, 0, nh))
                    pts = {0: emit_S(pair, nh, 0)}
                    for km in range(NO):
                        exp_done = clocks["act"]  # exp(km) finish frontier
                        if km + 1 < NO:
                            if km + 1 == 4:
                                force(("qk", pair, 1, 1))
                            elastic()
                            pts[km + 1] = emit_S(pair, nh, km + 1)
                        force(("v", qt, km))
                        pt = pts.pop(km)
                        # A.V of km stalls until exp(km) is done
                        clocks["pe"] = max(clocks["pe"], exp_done)
                        nc.tensor.matmul(
                            avA[0 : DH + 1, :],
                            v_all[:, km, hA, :],
                            pt[:, 0:512],
                            start=(km == 0),
                            stop=(km == NO - 1),
                        )
                        nc.tensor.matmul(
                            avB[0 : DH + 1, :],
                            v_all[:, km, hB, :],
                            pt[:, 512:1024],
                            start=(km == 0),
                            stop=(km == NO - 1),
                        )
                        pe(2 * COST_AV)
                    # epilogue: 1/denominator via exp(-ln) on ACT, broadcast
                    # across partitions with a full-K selector matmul, DVE mul
                    for slot, av in ((0, avA), (1, avB)):
                        ln_row = work.tile([P, 512], f32, tag="ln_row")
                        nc.scalar.activation(
                            ln_row[DH : DH + 1, :], av[DH : DH + 1, :], LN
                        )
                        nc.scalar.activation(
                            rec_t[DH : DH + 1, slot, :],
                            ln_row[DH : DH + 1, :],
                            EXP,
                            scale=-1.0,
                        )
                    clocks["act"] += COST_EPI_ACT
                    # fill until the reciprocal is ready, then finish any
                    # half-open unit so bc's mmp slot frees in order
                    elastic(1e9)
                    close_open()
                    clocks["pe"] = max(clocks["pe"], clocks["act"])
                    for slot, head, av in ((0, hA, avA), (1, hB, avB)):
                        bc = mmp.tile([DH, 512], f32, tag="mm", name="bc")
                        nc.tensor.matmul(bc, sel_t, rec_t[:, slot, :])
                        pe(COST_MM512)
                        # DVE can't read two PSUM operands; stage bc in SBUF
                        bc_sb = work.tile([DH, 512], bf16, tag="bc_sb")
                        nc.vector.tensor_copy(bc_sb, bc)
                        if head % 2 == 0:
                            dst = outT[0:DH, pair, nsl]
                        else:
                            dst = stage_odd[:, nsl]
                        nc.vector.tensor_mul(dst, av[0:DH, :], bc_sb)
                    # move this half's odd head to partitions 64:128 now so
                    # proj of these token chunks can start (pair 7 overlap)
                    nc.sync.dma_start(
                        out=outT[DH:P, pair, nsl],
                        in_=stage_odd[:, nsl],
                    )
                    if pair == NPAIR - 2:
                        # pairs 0-6 outT complete for this half's chunks:
                        # the pairs-0-6 projection partial becomes fill work
                        # (supply for the otherwise-starved pairs 6-7)
                        for no in range(nh * 4, nh * 4 + 4):
                            fillq.append(
                                (("part", no), g_partial(no), pair)
                            )
                    elif pair == NPAIR - 1:
                        # pair 7 done for this half: finish those chunks
                        for no in range(nh * 4, nh * 4 + 4):
                            fillq.append(
                                (("rem", no), g_remainder(no), pair)
                            )

            # ---------------- schedule ----------------
            # ramp: x chunks 0-3 -> transposes -> pair-0 nh0 qT/kT.
            # gpsimd DMA queue order: x0-3 interleaved with just the weight
            # slices the first matmuls need (x gates the whole ramp).
            # chunk no+4's DMA is emitted after chunk no's transposes so the
            # 4-slot ring's write order matches read order.
            x_dma(0)
            x_dma(1)
            nc.gpsimd.dma_start(out=wq[:, :, 0:P], in_=wqkv_src[:, :, 0:P])
            x_dma(2)
            nc.gpsimd.dma_start(
                out=wk[:, :, 0:P], in_=wqkv_src[:, :, C : C + P]
            )
            x_dma(3)
            # v weights for heads 0-3 (quarter 0), needed by pair 0's A.V
            nc.gpsimd.dma_start(
                out=wv[:, :, 0:256], in_=wqkv_src[:, :, 2 * C : 2 * C + 256]
            )
            for no in range(4):
                for c in g_transpose(no):
                    pe(c)
                x_dma(no + 4)
            for c in g_qk(0, 1, 0):
                pe(c)
            for c in g_qk(0, 0, 0):
                pe(c)
            done.update({("qk", 0, 1, 0), ("qk", 0, 0, 0)})

            # fill queue in dependency order (head-only draining keeps at
            # most one group open in the mmp ring -> no deadlock); the tag
            # is the earliest pair allowed to consume the unit elastically
            for km in range(4):
                fillq.append((("v", 0, km), g_v(km, 0), -1))
            for no in range(4, NO):
                fillq.append((("tr", no), g_transpose(no), -1))
            for km in range(4, NO):
                fillq.append((("v", 0, km), g_v(km, 0), -1))
            fillq.append((("qk", 0, 1, 1), g_qk(0, 1, 1), -1))
            fillq.append((("qk", 0, 0, 1), g_qk(0, 0, 1), -1))

            def queue_pair_qk(p):
                for which in (1, 0):
                    for nh in range(2):
                        fillq.append(
                            (("qk", p, which, nh), g_qk(p, which, nh), p - 1)
                        )

            queue_pair_qk(1)
            for km in range(NO):
                fillq.append((("v", 1, km), g_v(km, 1), 1))
            queue_pair_qk(2)
            queue_pair_qk(3)
            for km in range(NO):
                fillq.append((("v", 2, km), g_v(km, 2), 3))
            queue_pair_qk(4)
            queue_pair_qk(5)
            for km in range(NO):
                fillq.append((("v", 3, km), g_v(km, 3), 5))
            queue_pair_qk(6)
            queue_pair_qk(7)

            # remaining weight DMAs once the ramp-critical ones are queued,
            # in need order: wv quarter 1 (pairs 2-3), bias, q/k remainders
            # (pairs 1+), wv half 2 (pairs 4+)
            nc.gpsimd.dma_start(
                out=wv[:, :, 256:512],
                in_=wqkv_src[:, :, 2 * C + 256 : 2 * C + 512],
            )
            pb_ap = pb_ext[:]
            pb_src = bass.AP(
                tensor=pb_ap.tensor,
                offset=pb_ap.offset,
                ap=[[0, P], pb_ap.ap[0]],
            )
            nc.gpsimd.dma_start(out=pb, in_=pb_src)
            nc.gpsimd.dma_start(out=wq[:, :, P:C], in_=wqkv_src[:, :, P:C])
            nc.gpsimd.dma_start(
                out=wk[:, :, P:C], in_=wqkv_src[:, :, C + P : 2 * C]
            )
            nc.gpsimd.dma_start(
                out=wv[:, :, 512:1024],
                in_=wqkv_src[:, :, 2 * C + 512 : 3 * C],
            )

            for pair in range(NPAIR):
                if pair == 3:
                    # proj weights only needed at the tail; load mid-flight
                    nc.gpsimd.dma_start(
                        out=wproj,
                        in_=wproj_ext[:, :].rearrange("(o p) j -> p o j", p=P),
                    )
                attention(pair)

            # tail: whatever fills remain (projection remainders)
            while fillq:
                drain(1e9, respect_tags=False)

    nc.compile()
    return nc


def _get_nc():
    if "nc" not in _CACHE:
        _CACHE["nc"] = build_nc()
    return _CACHE["nc"]


def kernel(**inputs) -> np.ndarray:
    """Full-input entry point: shards batch over 8 cores, returns [8,N,C]."""
    from concourse.bass_utils import run_bass_kernel_spmd

    x = np.asarray(inputs["x"], dtype=np.float32)
    qkv_w = np.asarray(inputs["qkv_w"], dtype=np.float32)
    proj_w = np.asarray(inputs["proj_w"], dtype=np.float32)
    proj_b = np.asarray(inputs["proj_b"], dtype=np.float32)
    B = x.shape[0]
    assert B == 8, f"kernel hardcoded for B=8, got {B}"

    nc = _get_nc()
    in_maps = [
        {"x": x[i], "qkv_w": qkv_w, "proj_w": proj_w, "proj_b": proj_b}
        for i in range(B)
    ]
    res = run_bass_kernel_spmd(nc, in_maps, core_ids=list(range(B)))
    out = np.stack([res.results[i]["out"] for i in range(B)], axis=0)
    return out.astype(np.float32)


# revision 38
# speedup vs baseline: 1.1940x; 1.0078x over previous
"""Trainium2 Bass kernel: 16-head self-attention block (B=8, N=1024, C=1024).

Data-parallel over batch: each of the 8 NeuronCores processes one batch
element end-to-end (QKV proj -> attention -> softmax -> out proj). No
collectives. Compute in bf16 (fp32 PSUM accumulation).

v19 (from v17 382us -> v18 379us): elastic fill scheduling with virtual
PE/ACT clocks — drains a fill queue (x-transposes, next pairs' qkT
groups, v quarter-groups, projection partials) one matmul at a time
whenever the PE clock falls behind the ACT clock. v18 traces showed the
dense schedule runs the whole chip in the P0 power state (PE 2.0 GHz,
ACT 1.0 GHz; N=512 matmul = 259 ns not 216), the ramp was DMA-starved
(x chunks queued behind 2 MB of wv on the gpsimd queue), fills ran out
during pairs 5-7, and the tail held the full output projection. v19:
costs calibrated to P0 clocks; gpsimd DMA queue reordered x-first;
fill rationing (per-unit earliest-pair tags) so late pairs keep supply;
projection split into a pairs-0-6 partial (fill work for pairs 6-7,
staged +bias in SBUF) and a tiny pair-7 remainder at the tail.
Other v18 changes kept: x DMA'd with f32->bf16 cast and transposed in
bf16 (4 transposes per PSUM tile + one DVE copy), v in N=256 quarter
groups JIT, per-nh stage_odd DMAs.

Layout (unchanged from v17): xT via PE identity-transpose; qT/kT per
head-pair [128, n] w-stationary; scores^T row-packed (two K=64 heads
concurrent via tile_position); exp on ACT (scores O(1), no max-sub);
A.V with v|ones (denominator lands in PSUM row 64); 1/s = exp(-ln s) on
ACT sharing one table set; partition-broadcast via selector matmul.
"""

import sys

sys.path.insert(0, "/opt/trn_rl_repo")

from collections import deque

import numpy as np

P = 128
N = 1024  # tokens
C = 1024  # channels
H = 16  # heads
DH = 64  # head dim
NPAIR = 8  # head pairs
CO = C // P  # 8 outer chunks of contraction dim
NO = N // P  # 8 outer chunks of token dim
SCALE = DH ** -0.5
KERNEL_VERSION = 20  # bump on every semantic change (busts stale NEFF caches)

# virtual-clock cost estimates (ns) for the elastic scheduler, calibrated
# on HW traces at the P0 power-state clocks the dense schedule runs at
# (PE ~2.0 GHz, ACT ~1.0 GHz): N=512 matmul streams at ~259 ns, the
# score pair pays an extra unhidden kT LDWEIGHTS (~390 ns total), exp of
# [128,1024] is ~1340 ns.
COST_MM512 = 260
COST_MM256 = 136
COST_TR = 128  # 128x128 bf16 transpose
COST_SPAIR = 390  # row-packed score pair incl. kT weight load
COST_AV = 260
COST_EXP = 1340  # ACT exp on [128,1024] f32->bf16
COST_EPI_ACT = 3350  # ACT ln+exp for both heads

_CACHE = {}


def build_nc():
    import concourse.bass as bass
    import concourse.tile as tile
    from concourse import bacc, masks, mybir

    # Route Exp to natural_log_exp_and_others (which also holds Ln) so the
    # exp(-ln(s)) reciprocal shares one ACT table set with the softmax exp.
    if not getattr(bacc, "_exp_ln_patch", False):
        _orig_tables = bacc.get_activation_tables

        def _patched_tables(arch):
            t = _orig_tables(arch)
            for name, fns in t.items():
                if name != "natural_log_exp_and_others":
                    fns.discard(mybir.ActivationFunctionType.Exp)
            return t

        bacc.get_activation_tables = _patched_tables
        bacc._exp_ln_patch = True

    f32 = mybir.dt.float32
    bf16 = mybir.dt.bfloat16
    EXP = mybir.ActivationFunctionType.Exp
    LN = mybir.ActivationFunctionType.Ln
    COPY = mybir.ActivationFunctionType.Copy

    nc = bacc.Bacc(None, target_bir_lowering=False)

    x_ext = nc.declare_dram_parameter("x", [N, C], f32, isOutput=False)
    wqkv_ext = nc.declare_dram_parameter("qkv_w", [C, 3 * C], f32, isOutput=False)
    wproj_ext = nc.declare_dram_parameter("proj_w", [C, C], f32, isOutput=False)
    pb_ext = nc.declare_dram_parameter("proj_b", [C], f32, isOutput=False)
    out_ext = nc.declare_dram_parameter("out", [N, C], f32, isOutput=True)
    # tiny version-stamped output: busts any executable cache keyed on the
    # HLO signature, and lets the harness confirm which kernel build ran
    ver_ext = nc.declare_dram_parameter(
        "kver", [1, KERNEL_VERSION], f32, isOutput=True
    )

    with tile.TileContext(nc) as tc:
        with (
            tc.tile_pool(name="big", bufs=1) as big,
            tc.tile_pool(name="work", bufs=3) as work,
            tc.tile_pool(name="xfp", bufs=2) as xfp,
            tc.tile_pool(name="xbp", bufs=4) as xbp,
            tc.tile_pool(name="ptp", bufs=4) as ptp,
            tc.tile_pool(name="mmp", bufs=2, space="PSUM") as mmp,
            tc.tile_pool(name="spool", bufs=2, space="PSUM") as spool,
            tc.tile_pool(name="avp", bufs=1, space="PSUM") as avp,
        ):
            # ---------------- constants / big buffers ----------------
            wq = big.tile([P, CO, C], bf16, tag="wq")
            wk = big.tile([P, CO, C], bf16, tag="wk")
            wv = big.tile([P, CO, C], bf16, tag="wv")
            wproj = big.tile([P, CO, C], bf16, tag="wproj")
            pb = big.tile([P, C], f32, tag="pb")
            xT = big.tile([P, CO, N], bf16, tag="xT")
            v_all = big.tile([P, NO, H, DH + 1], bf16, tag="v_all")
            qT = big.tile([P, NPAIR, N], bf16, tag="qT")
            kT = big.tile([P, NPAIR, N], bf16, tag="kT")
            outT = big.tile([P, NPAIR, N], bf16, tag="outT")
            # single plane, reused per pair (DMA'd to outT within the pair)
            stage_odd = big.tile([DH, N], bf16, tag="stage_odd")
            ident = big.tile([P, P], bf16, tag="ident")
            # selector for the partition-broadcast matmul: row 64 ones
            sel_t = big.tile([P, DH], bf16, tag="sel_t")
            # persistent reciprocal staging, 2 slots (head A / head B) so the
            # two bc matmuls never WAR-stall the ACT; rows != 64 stay at 1.0
            # so the full-K broadcast matmul never touches uninitialized data
            rec_t = big.tile([P, 2, 512], bf16, tag="rec_t")
            # pairs 0-6 projection partials + bias, staged for the tail
            partial_sb = big.tile([P, NO, 2, 512], bf16, tag="partial_sb")

            ver_sb = big.tile([1, KERNEL_VERSION], f32, tag="ver_sb")
            nc.vector.memset(ver_sb, float(KERNEL_VERSION))
            nc.sync.dma_start(out=ver_ext[:, :], in_=ver_sb)
            # ones column of v|ones
            nc.vector.memset(v_all[:, :, :, DH : DH + 1], 1.0)
            nc.vector.memset(sel_t, 0.0)
            nc.vector.memset(sel_t[DH : DH + 1, :], 1.0)
            nc.vector.memset(rec_t, 1.0)
            masks.make_identity(nc, ident)

            # ---------------- input DMAs ----------------
            # all casting DMAs go through the gpsimd queue; order matters:
            # x chunks 0-3 first (they gate the whole ramp), then just the
            # weight slices the first matmuls need
            wqkv_src = wqkv_ext[:, :].rearrange("(o p) j -> p o j", p=P)

            # x chunks: f32 DMA on the sync queue (parallel with the weight
            # DMAs on gpsimd), cast f32->bf16 on the ACT engine (idle during
            # the ramp), then transpose in bf16 on the PE
            xbs = {}

            def x_dma(no):
                # 4-slot rings: chunk no+4's DMA waits until chunk no's
                # cast/transposes consumed its slots (ramp does 0-3 eagerly)
                xf = xfp.tile([P, C], f32, tag="xf")
                nc.sync.dma_start(out=xf, in_=x_ext[no * P : (no + 1) * P, :])
                xb = xbp.tile([P, C], bf16, tag="xb")
                # Copy lives in every ACT table set: no table-swap cost
                nc.scalar.activation(xb, xf, COPY)
                xbs[no] = xb

            # ---------------- elastic fill scheduler ----------------
            clocks = {"pe": 0.0, "act": 0.0}
            cur = {"pair": -1}
            done = set()
            started = set()
            fillq = deque()  # (key, generator, min_pair) — head-only drain

            def pe(ns):
                clocks["pe"] += ns

            def _step():
                """Advance the head fill unit by one instruction."""
                key, g, _tag = fillq[0]
                started.add(key)
                c = next(g, None)
                if c is None:
                    done.add(key)
                    fillq.popleft()
                    return 0.0
                pe(c)
                return c

            def drain(budget, respect_tags=False):
                """Drain fills from the queue head. (A tag-based rationing
                experiment lost more to blocked-while-idle stalls than late
                starvation cost — the pairs-0-6 projection partials already
                supply pairs 6-7, so rationing stays off.)"""
                spent = 0.0
                while fillq and spent < budget:
                    spent += _step()
                return spent

            def force(key):
                while key not in done:
                    _step()

            def close_open():
                """Finish a half-emitted fill unit so its mmp ring slots
                free in emission order (deadlock safety before bc/proj)."""
                if fillq and fillq[0][0] in started:
                    k = fillq[0][0]
                    force(k)

            def elastic(cap=1400.0):
                gap = clocks["act"] - clocks["pe"]
                if gap > 0:
                    drain(min(gap, cap))

            # ---------------- fill unit generators ----------------
            def g_transpose(no):
                """Transpose x chunk no: 2 PSUM groups of 4, 1 copy each."""
                xb = xbs[no]
                for g4 in range(2):
                    # transpose-mode out dtype must match lhsT dtype (bf16)
                    ps = mmp.tile([P, 512], bf16, tag="mm", name="pst")
                    for i in range(4):
                        co = g4 * 4 + i
                        nc.tensor.transpose(
                            ps[:, i * P : (i + 1) * P],
                            xb[:, co * P : (co + 1) * P],
                            ident,
                        )
                        if not (i == 3):
                            yield COST_TR
                    nc.vector.tensor_copy(
                        xT[:, g4 * 4 : g4 * 4 + 4, no * P : (no + 1) * P],
                        ps[:].rearrange("p (c q) -> p c q", c=4),
                    )
                    yield COST_TR

            def g_qk(pair, which, nh):
                """One q^T/k^T half: 8 accumulating matmuls + copy-out."""
                w = wq if which == 0 else wk
                dst = qT if which == 0 else kT
                nsl = slice(nh * 512, (nh + 1) * 512)
                ps = mmp.tile([P, 512], f32, tag="mm", name="ps")
                for co in range(CO):
                    nc.tensor.matmul(
                        ps,
                        w[:, co, pair * P : (pair + 1) * P],
                        xT[:, co, nsl],
                        start=(co == 0),
                        stop=(co == CO - 1),
                    )
                    if co < CO - 1:
                        yield COST_MM512
                if which == 0:
                    # fold softmax scale into q
                    nc.vector.tensor_scalar_mul(dst[:, pair, nsl], ps, SCALE)
                else:
                    nc.vector.tensor_copy(dst[:, pair, nsl], ps)
                yield COST_MM512

            def g_v(km, qt):
                """v columns for heads 4qt..4qt+4, token chunk km."""
                ps = mmp.tile([P, 256], f32, tag="mm", name="psv")
                for co in range(CO):
                    nc.tensor.matmul(
                        ps,
                        xT[:, co, km * P : (km + 1) * P],
                        wv[:, co, qt * 256 : (qt + 1) * 256],
                        start=(co == 0),
                        stop=(co == CO - 1),
                    )
                    if co < CO - 1:
                        yield COST_MM256
                nc.vector.tensor_copy(
                    v_all[:, km, qt * 4 : (qt + 1) * 4, 0:DH],
                    ps[:].rearrange("p (h d) -> p h d", h=4),
                )
                yield COST_MM256

            def g_partial(no):
                """Output projection for token chunk no, pairs 0-6 only.
                Runs as fill work once pairs 0-6's outT covers this chunk
                (mid pair-6); result + bias staged in SBUF so only pair 7's
                single accumulation step remains at the tail."""
                ps0 = mmp.tile([P, 512], f32, tag="mm", name="ps0")
                ps1 = mmp.tile([P, 512], f32, tag="mm", name="ps1")
                for pair in range(NPAIR - 1):
                    lhs = outT[:, pair, no * P : (no + 1) * P]
                    nc.tensor.matmul(
                        ps0, lhs, wproj[:, pair, 0:512],
                        start=(pair == 0), stop=(pair == NPAIR - 2),
                    )
                    yield COST_MM512
                    nc.tensor.matmul(
                        ps1, lhs, wproj[:, pair, 512:1024],
                        start=(pair == 0), stop=(pair == NPAIR - 2),
                    )
                    if pair < NPAIR - 2:
                        yield COST_MM512
                for jh, ps in ((0, ps0), (1, ps1)):
                    nc.vector.tensor_add(
                        partial_sb[:, no, jh, :],
                        ps,
                        pb[:, jh * 512 : (jh + 1) * 512],
                    )
                yield COST_MM512

            def g_remainder(no):
                """Tail of the output projection for chunk no: pair 7's
                contribution + staged partial, then DMA out."""
                ps0 = mmp.tile([P, 512], f32, tag="mm", name="ps0")
                ps1 = mmp.tile([P, 512], f32, tag="mm", name="ps1")
                lhs = outT[:, NPAIR - 1, no * P : (no + 1) * P]
                nc.tensor.matmul(ps0, lhs, wproj[:, NPAIR - 1, 0:512])
                yield COST_MM512
                nc.tensor.matmul(ps1, lhs, wproj[:, NPAIR - 1, 512:1024])
                for jh, ps in ((0, ps0), (1, ps1)):
                    res = work.tile([P, 512], f32, tag="res")
                    nc.vector.tensor_add(res, ps, partial_sb[:, no, jh, :])
                    nc.sync.dma_start(
                        out=out_ext[
                            no * P : (no + 1) * P, jh * 512 : (jh + 1) * 512
                        ],
                        in_=res,
                    )
                yield COST_MM512

            # ---------------- attention ----------------
            def emit_S(pair, nh, km):
                """scores^T for both heads of `pair`: row-packed matmuls,
                then the exp on ACT. Returns the pt tile."""
                nsl = slice(nh * 512, (nh + 1) * 512)
                s = spool.tile([P, N], f32, tag="S")
                nc.tensor.matmul(
                    s[:, 0:512],
                    kT[0:DH, pair, km * P : (km + 1) * P],
                    qT[0:DH, pair, nsl],
                )
                nc.tensor.matmul(
                    s[:, 512:1024],
                    kT[DH:P, pair, km * P : (km + 1) * P],
                    qT[DH:P, pair, nsl],
                    tile_position=(DH, 0),
                )
                pe(COST_SPAIR)  # concurrent pair + kT weight load
                pt = ptp.tile([P, N], bf16, tag="pt")
                nc.scalar.activation(pt, s, EXP)
                clocks["act"] = max(clocks["act"], clocks["pe"]) + COST_EXP
                return pt

            def attention(pair):
                cur["pair"] = pair
                qt = pair // 2
                hA, hB = 2 * pair, 2 * pair + 1
                for nh in range(2):
                    nsl = slice(nh * 512, (nh + 1) * 512)
                    avA = avp.tile([P, 512], f32, tag="avA")
                    avB = avp.tile([P, 512], f32, tag="avB")
                    force(("qk", pair, 1, 0))
                    force(("qk", pair, 0, nh))
                    pts = {0: emit_S(pair, nh, 0)}
                    for km in range(NO):
                        exp_done = clocks["act"]  # exp(km) finish frontier
                        if km + 1 < NO:
                            if km + 1 == 4:
                                force(("qk", pair, 1, 1))
                            elastic()
                            pts[km + 1] = emit_S(pair, nh, km + 1)
                        force(("v", qt, km))
                        pt = pts.pop(km)
                        # A.V of km stalls until exp(km) is done
                        clocks["pe"] = max(clocks["pe"], exp_done)
                        nc.tensor.matmul(
                            avA[0 : DH + 1, :],
                            v_all[:, km, hA, :],
                            pt[:, 0:512],
                            start=(km == 0),
                            stop=(km == NO - 1),
                        )
                        nc.tensor.matmul(
                            avB[0 : DH + 1, :],
                            v_all[:, km, hB, :],
                            pt[:, 512:1024],
                            start=(km == 0),
                            stop=(km == NO - 1),
                        )
                        pe(2 * COST_AV)
                    # epilogue: 1/denominator via exp(-ln) on ACT, broadcast
                    # across partitions with a full-K selector matmul, DVE mul
                    for slot, av in ((0, avA), (1, avB)):
                        ln_row = work.tile([P, 512], f32, tag="ln_row")
                        nc.scalar.activation(
                            ln_row[DH : DH + 1, :], av[DH : DH + 1, :], LN
                        )
                        nc.scalar.activation(
                            rec_t[DH : DH + 1, slot, :],
                            ln_row[DH : DH + 1, :],
                            EXP,
                            scale=-1.0,
                        )
                    clocks["act"] += COST_EPI_ACT
                    # fill until the reciprocal is ready, then finish any
                    # half-open unit so bc's mmp slot frees in order
                    elastic(1e9)
                    close_open()
                    clocks["pe"] = max(clocks["pe"], clocks["act"])
                    for slot, head, av in ((0, hA, avA), (1, hB, avB)):
                        bc = mmp.tile([DH, 512], f32, tag="mm", name="bc")
                        nc.tensor.matmul(bc, sel_t, rec_t[:, slot, :])
                        pe(COST_MM512)
                        # DVE can't read two PSUM operands; stage bc in SBUF
                        bc_sb = work.tile([DH, 512], bf16, tag="bc_sb")
                        nc.vector.tensor_copy(bc_sb, bc)
                        if head % 2 == 0:
                            dst = outT[0:DH, pair, nsl]
                        else:
                            dst = stage_odd[:, nsl]
                        nc.vector.tensor_mul(dst, av[0:DH, :], bc_sb)
                    # move this half's odd head to partitions 64:128 now so
                    # proj of these token chunks can start (pair 7 overlap)
                    nc.sync.dma_start(
                        out=outT[DH:P, pair, nsl],
                        in_=stage_odd[:, nsl],
                    )
                    if pair == NPAIR - 2:
                        # pairs 0-6 outT complete for this half's chunks:
                        # the pairs-0-6 projection partial becomes fill work
                        # (supply for the otherwise-starved pairs 6-7)
                        for no in range(nh * 4, nh * 4 + 4):
                            fillq.append(
                                (("part", no), g_partial(no), pair)
                            )
                    elif pair == NPAIR - 1:
                        # pair 7 done for this half: finish those chunks
                        for no in range(nh * 4, nh * 4 + 4):
                            fillq.append(
                                (("rem", no), g_remainder(no), pair)
                            )

            # ---------------- schedule ----------------
            # ramp: x chunks 0-3 -> transposes -> pair-0 nh0 qT/kT.
            # gpsimd DMA queue order: x0-3 interleaved with just the weight
            # slices the first matmuls need (x gates the whole ramp).
            # chunk no+4's DMA is emitted after chunk no's transposes so the
            # 4-slot ring's write order matches read order.
            x_dma(0)
            x_dma(1)
            nc.gpsimd.dma_start(out=wq[:, :, 0:P], in_=wqkv_src[:, :, 0:P])
            x_dma(2)
            nc.gpsimd.dma_start(
                out=wk[:, :, 0:P], in_=wqkv_src[:, :, C : C + P]
            )
            x_dma(3)
            # v weights for heads 0-3 (quarter 0), needed by pair 0's A.V
            nc.gpsimd.dma_start(
                out=wv[:, :, 0:256], in_=wqkv_src[:, :, 2 * C : 2 * C + 256]
            )
            for no in range(4):
                for c in g_transpose(no):
                    pe(c)
                x_dma(no + 4)
            for c in g_qk(0, 1, 0):
                pe(c)
            for c in g_qk(0, 0, 0):
                pe(c)
            done.update({("qk", 0, 1, 0), ("qk", 0, 0, 0)})

            # fill queue in dependency order (head-only draining keeps at
            # most one group open in the mmp ring -> no deadlock); the tag
            # is the earliest pair allowed to consume the unit elastically
            for km in range(4):
                fillq.append((("v", 0, km), g_v(km, 0), -1))
            for no in range(4, NO):
                fillq.append((("tr", no), g_transpose(no), -1))
            for km in range(4, NO):
                fillq.append((("v", 0, km), g_v(km, 0), -1))
            fillq.append((("qk", 0, 1, 1), g_qk(0, 1, 1), -1))
            fillq.append((("qk", 0, 0, 1), g_qk(0, 0, 1), -1))

            def queue_pair_qk(p):
                for which in (1, 0):
                    for nh in range(2):
                        fillq.append(
                            (("qk", p, which, nh), g_qk(p, which, nh), p - 1)
                        )

            queue_pair_qk(1)
            for km in range(NO):
                fillq.append((("v", 1, km), g_v(km, 1), 1))
            queue_pair_qk(2)
            queue_pair_qk(3)
            for km in range(NO):
                fillq.append((("v", 2, km), g_v(km, 2), 3))
            queue_pair_qk(4)
            queue_pair_qk(5)
            for km in range(NO):
                fillq.append((("v", 3, km), g_v(km, 3), 5))
            queue_pair_qk(6)
            queue_pair_qk(7)

            # remaining weight DMAs once the ramp-critical ones are queued,
            # in need order: wv quarter 1 (pairs 2-3), bias, q/k remainders
            # (pairs 1+), wv half 2 (pairs 4+)
            nc.gpsimd.dma_start(
                out=wv[:, :, 256:512],
                in_=wqkv_src[:, :, 2 * C + 256 : 2 * C + 512],
            )
            pb_ap = pb_ext[:]
            pb_src = bass.AP(
                tensor=pb_ap.tensor,
                offset=pb_ap.offset,
                ap=[[0, P], pb_ap.ap[0]],
            )
            nc.gpsimd.dma_start(out=pb, in_=pb_src)
            nc.gpsimd.dma_start(out=wq[:, :, P:C], in_=wqkv_src[:, :, P:C])
            nc.gpsimd.dma_start(
                out=wk[:, :, P:C], in_=wqkv_src[:, :, C + P : 2 * C]
            )
            nc.gpsimd.dma_start(
                out=wv[:, :, 512:1024],
                in_=wqkv_src[:, :, 2 * C + 512 : 3 * C],
            )

            for pair in range(NPAIR):
                if pair == 3:
                    # proj weights only needed at the tail; load mid-flight
                    nc.gpsimd.dma_start(
                        out=wproj,
                        in_=wproj_ext[:, :].rearrange("(o p) j -> p o j", p=P),
                    )
                attention(pair)

            # tail: whatever fills remain (projection remainders)
            while fillq:
                drain(1e9, respect_tags=False)

    nc.compile()
    return nc


def _get_nc():
    if "nc" not in _CACHE:
        _CACHE["nc"] = build_nc()
    return _CACHE["nc"]


def kernel(**inputs) -> np.ndarray:
    """Full-input entry point: shards batch over 8 cores, returns [8,N,C]."""
    from concourse.bass_utils import run_bass_kernel_spmd

    x = np.asarray(inputs["x"], dtype=np.float32)
    qkv_w = np.asarray(inputs["qkv_w"], dtype=np.float32)
    proj_w = np.asarray(inputs["proj_w"], dtype=np.float32)
    proj_b = np.asarray(inputs["proj_b"], dtype=np.float32)
    B = x.shape[0]
    assert B == 8, f"kernel hardcoded for B=8, got {B}"

    nc = _get_nc()
    in_maps = [
        {"x": x[i], "qkv_w": qkv_w, "proj_w": proj_w, "proj_b": proj_b}
        for i in range(B)
    ]
    res = run_bass_kernel_spmd(nc, in_maps, core_ids=list(range(B)))
    out = np.stack([res.results[i]["out"] for i in range(B)], axis=0)
    return out.astype(np.float32)
